# revision 17
# baseline (speedup 1.0000x reference)
"""Trainium2 Bass kernel for nn_BaselineAttention_25984552141259.

Problem: QKV [3, B=2, H=8, N=4096, d=64] fp32 ->
         out[b,h,n,:] = softmax(Q[b,h] @ K[b,h].T) @ V[b,h]

Sharding: B*H = 16 heads, embarrassingly parallel -> 2 heads per core on 8
NeuronCores.

Numerics: scores need fp32 (bf16 Q*K alone costs 2e-2 max-rel-err -- over the
gate), so the S matmul runs fp32r. P and V run bf16: the P*V error is tiny
(softmax weights are insensitive to 0.4% V noise) and bf16 halves the exp
write cost + V DMA.

Device algorithm per head (flash-attention style, S^T layout), processed in
chunk-PAIRS of 1024 query columns:

  S^T[m, n]  = sum_d K^T[d, m] * Q^T[d, n]    (PE, fp32r; K^T zero-padded
                                               d=64 -> 128 partitions ON
                                               DEVICE via one-time pad
                                               tensor_copies -- K=64 matmuls
                                               never un-throttle the PE HAM
                                               clock gate, and unpadded DMA
                                               halves Q/K bytes)
  P^T = exp(S^T - 18)                          (constant bias instead of row
                                               max: scores ~ N(0,64), fp32
                                               psum can't overflow; softmax is
                                               shift-invariant. Split across
                                               TWO engines so neither paces
                                               the PE:
                                               - ACT: spline Exp -> bf16,
                                                 25/32 m-blocks
                                               - DVE: Schraudolph exp2 bit
                                                 trick 7/32:
                                                 bf16_bits = trunc(128*log2e*s
                                                 + C) as int16, bitcast bf16;
                                                 balanced magic constant keeps
                                                 rel err within +-3%)
  O^T[d', n] = sum_m V'[m, d'] * P^T[m, n]     (PE, bf16, accumulate over m;
                                               row d'=64 = softmax denominator
                                               via the host-appended ones col)
  out^T      = O^T[0:64] / bcast(O^T[64])      (DVE copy frees the psum
                                               accumulator in ~1.2us; the
                                               8-cycle/elem DVE reciprocal is
                                               split into 4x256 pieces DEFERRED
                                               into the next pair's DVE stream
                                               so it never stalls the exp
                                               pipeline; denominator broadcast
                                               via DRAM-bounce stride-0 DMA;
                                               final multiply on GPSIMD.
                                               The very last pair ships the
                                               raw [O|den] tile instead and the
                                               host divides during unshard --
                                               saves ~9us of serial drain.)
Host re-transposes out^T -> [N, d] while unsharding.
"""
import numpy as np
import ml_dtypes
from contextlib import ExitStack

import concourse.bass as bass
import concourse.tile as tile
from concourse import bacc, mybir
from concourse.bass_utils import run_bass_kernel_spmd

N_CORES = 8
B, H, N, D = 2, 8, 4096, 64
HEADS = B * H
HPC = HEADS // N_CORES          # heads per core = 2
NCHUNK = 512                    # columns per matmul (ISA moving-dim cap)
NPAIR = 2 * NCHUNK              # chunk-pair processed per stationary reload
PAIRS = N // NPAIR              # 4 pairs per head
MB = N // 128                   # 32 m-blocks of 128 keys
EXP_BIAS = 18.0                 # P = exp(s - 18); s in ~[-60, 60]
LOG2E = 1.4426950408889634

# DVE exp2 bit trick: bf16(2^t) bits ~= 128*(t + 127 - c), c = 0.043 balances
# the (1+f) vs 2^f linear-mantissa error to +-3%; +0.5 compensates the
# truncating fp32->int16 convert. t = (s - EXP_BIAS)*log2e.
TS_SCALE = 128.0 * LOG2E
TS_OFF = 128.0 * (127.0 - 0.043 - EXP_BIAS * LOG2E) + 0.5
# exp stages on DVE instead of ACT; start at m=4 so the pair-boundary copy
# never delays an O-matmul, end before m=30 so the copy isn't queued behind
# a boundary exp
DVE_STAGES = frozenset(m for m in range(MB) if m % 3 == 1 and 4 <= m < 30)

F32 = mybir.dt.float32
F32R = mybir.dt.float32r
BF16 = mybir.dt.bfloat16
I16 = mybir.dt.int16

KT_PIECES0 = [1, 1, 2, 4, 8, 16]
KT_PIECES1 = [16, 16]

_CACHE = {}


def _build():
    nc = bacc.Bacc("TRN2", target_bir_lowering=False, debug=False,
                   num_devices=N_CORES)
    # fp32r DRAM view: the PE's fp32r path reads a rounded fp32 payload;
    # declaring the tensor fp32r lets a plain DMA feed the fp32r matmul.
    qt_d = nc.dram_tensor("qt", [HPC, D, N], F32R, kind="ExternalInput").ap()
    kt_d = nc.dram_tensor("kt", [HPC, D, N], F32R, kind="ExternalInput").ap()
    v_d = nc.dram_tensor("v", [HPC, 128, MB, D + 1], BF16,
                         kind="ExternalInput").ap()
    ot_d = nc.dram_tensor("ot", [HPC, D, N], F32, kind="ExternalOutput").ap()
    # raw [O | den] of the final pair; host performs that divide
    otail_d = nc.dram_tensor("otail", [D + 1, NPAIR], F32,
                             kind="ExternalOutput").ap()

    with tile.TileContext(nc) as tc, ExitStack() as ctx:
        const = ctx.enter_context(tc.tile_pool(name="const", bufs=1))
        kq = ctx.enter_context(tc.tile_pool(name="kq", bufs=1))
        vpool = ctx.enter_context(tc.tile_pool(name="vpool", bufs=1))
        pexp = ctx.enter_context(tc.tile_pool(name="pexp", bufs=4))
        ocpool = ctx.enter_context(tc.tile_pool(name="ocpool", bufs=2))
        opool = ctx.enter_context(tc.tile_pool(name="opool", bufs=2))
        bcpool = ctx.enter_context(tc.tile_pool(name="bcpool", bufs=2))
        rpool = ctx.enter_context(tc.tile_pool(name="rpool", bufs=2))
        s_ps = ctx.enter_context(tc.tile_pool(name="s_ps", bufs=3, space="PSUM"))
        ot_ps = ctx.enter_context(tc.tile_pool(name="ot_ps", bufs=1, space="PSUM"))
        rdram = ctx.enter_context(tc.tile_pool(name="rdram", bufs=2, space="DRAM"))

        bias_t = const.tile([128, 1], F32)
        nc.vector.memset(bias_t[:], -EXP_BIAS)
        zero_f = const.tile([128, 1], F32)
        nc.vector.memset(zero_f[:], 0.0)

        # ---- loads ----
        kt_tiles = [[None] * MB for _ in range(HPC)]
        v_tiles = [[None] * MB for _ in range(HPC)]
        qt_tiles = [[None] * PAIRS for _ in range(HPC)]

        def load_kt(h, m0, g, eng=None):
            t = kq.tile([128, g, 128], F32R, tag=f"kt_{h}_{m0}",
                        name=f"kt_{h}_{m0}")
            nc.vector.tensor_copy(
                t[D:128, :, :].rearrange("p t q -> p (t q)"),
                zero_f[0:128 - D, 0:1].to_broadcast((128 - D, g * 128)))
            (eng or nc.gpsimd).dma_start(
                t[0:D, :, :],
                kt_d[h, :, m0 * 128:(m0 + g) * 128].rearrange(
                    "p (t q) -> p t q", q=128))
            for j in range(g):
                kt_tiles[h][m0 + j] = t[:, j, :]

        def load_qt(h, pr, split=1, engs=None):
            t = kq.tile([128, 2, NCHUNK], F32R, tag=f"qt_{h}_{pr}",
                        name=f"qt_{h}_{pr}")
            nc.vector.tensor_copy(
                t[D:128, :, :].rearrange("p t q -> p (t q)"),
                zero_f[0:128 - D, 0:1].to_broadcast((128 - D, NPAIR)))
            w = NPAIR // split
            for i in range(split):
                eng = (engs or [nc.gpsimd])[i % len(engs or [nc.gpsimd])]
                eng.dma_start(
                    t[0:D, :, :].rearrange("p t q -> p (t q)")[:,
                        i * w:(i + 1) * w].rearrange("p (t q) -> p t q", q=w),
                    qt_d[h, :, pr * NPAIR + i * w:pr * NPAIR + (i + 1) * w
                         ].rearrange("p (t q) -> p t q", q=w))
            qt_tiles[h][pr] = t

        def load_v(h, m0, g, eng=None):
            t = vpool.tile([128, g, D + 1], BF16, tag=f"v_{h}_{m0}",
                           name=f"v_{h}_{m0}")
            (eng or nc.gpsimd).dma_start(t[:], v_d[h, :, m0:m0 + g, :])
            for j in range(g):
                v_tiles[h][m0 + j] = t[:, j, :]

        with nc.named_scope("load"):
            # first wave across four trigger engines: the first m-block,
            # q-pair and v-block land in parallel DMA queues within ~3us
            load_kt(0, 0, 1, eng=nc.gpsimd)
            load_qt(0, 0, split=4, engs=[nc.sync, nc.scalar])
            load_v(0, 0, 1, eng=nc.sync)
            load_kt(0, 1, 1, eng=nc.scalar)
            m0 = 2
            for i, g in enumerate([2, 4, 8, 16]):
                load_kt(0, m0, g)
                load_v(0, m0 - 1, g)        # v trails kt by one piece
                if i == 0:
                    load_qt(0, 1, split=2, engs=[nc.sync, nc.scalar])
                if i == 1:
                    load_qt(0, 2)
                if i == 2:
                    load_qt(0, 3)
                m0 += g
            load_v(0, m0 - 1, 1)
            m0 = 0
            for i, g in enumerate(KT_PIECES1):
                load_kt(1, m0, g)
                load_v(1, m0, g)
                load_qt(1, 2 * i)
                load_qt(1, 2 * i + 1)
                m0 += g

        # ---- compute ----
        # deferred normalize actions from the previous pair, keyed by the
        # stage index of the CURRENT pair at which each may be issued
        deferred = []

        def run_deferred(stage):
            while deferred and deferred[0][0] <= stage:
                deferred.pop(0)[1]()

        for h in range(HPC):
            with nc.named_scope(f"head{h}"):
                for pr in range(PAIRS):
                    qt_t = qt_tiles[h][pr]
                    last = (h == HPC - 1) and (pr == PAIRS - 1)
                    ot_t = ot_ps.tile([D + 1, 2, NCHUNK], F32, tag="ot",
                                      name=f"ot_{h}_{pr}")
                    for m in range(MB):
                        run_deferred(m)
                        s_t = s_ps.tile([128, 2, NCHUNK], F32, tag="s")
                        for j in range(2):
                            nc.tensor.matmul(s_t[:, j, :], kt_tiles[h][m],
                                             qt_t[:, j, :],
                                             start=True, stop=True)
                        p_t = pexp.tile([128, 2, NCHUNK], BF16, tag="p")
                        if m in DVE_STAGES:
                            nc.vector.tensor_scalar(
                                p_t[:].bitcast(I16), s_t[:],
                                TS_SCALE, TS_OFF,
                                mybir.AluOpType.mult, mybir.AluOpType.add)
                        else:
                            nc.scalar.activation(
                                p_t[:], s_t[:],
                                mybir.ActivationFunctionType.Exp,
                                bias=bias_t[:], scale=1.0)
                        for j in range(2):
                            nc.tensor.matmul(ot_t[:, j, :], v_tiles[h][m],
                                             p_t[:, j, :],
                                             start=(m == 0),
                                             stop=(m == MB - 1))
                    run_deferred(MB)

                    # free the single psum accumulator ASAP via a DVE copy
                    oc = ocpool.tile([D + 1, NPAIR], F32, tag="oc")
                    nc.vector.tensor_copy(
                        oc[:], ot_t[:].rearrange("p t q -> p (t q)"))
                    if last:
                        # raw [O | den] out; host divides during unshard
                        nc.sync.dma_start(otail_d[:, :], oc[:])
                        continue
                    # Reciprocal trick: the DVE iterative divide is ~8
                    # cycles per FREE element but parallel across
                    # partitions, so 1/den on the [1, 1024] row costs 6.5us
                    # while the same values transposed to [128, 8] cost
                    # ~0.3us. The DRAM bounce (needed for the partition
                    # broadcast anyway) provides the transpose for free.
                    # Everything below is deferred into the next pair's
                    # stream and runs off the critical path.
                    bc = bcpool.tile([D, NPAIR], F32, tag="bc")
                    o_t = opool.tile([D, NPAIR], F32, tag="o")
                    den_d = rdram.tile([1, NPAIR], F32, tag="den_d")
                    rec_d = rdram.tile([1, NPAIR], F32, tag="rec_d")
                    den_t = rpool.tile([128, NPAIR // 128], F32, tag="den_t")
                    rec_t = rpool.tile([128, NPAIR // 128], F32, tag="rec_t")
                    nc.sync.dma_start(den_d[:], oc[D:D + 1, :])
                    oh, opr = h, pr

                    def read_t(den_t=den_t, den_d=den_d):
                        nc.sync.dma_start(
                            den_t[:],
                            den_d[:].rearrange("o (p f) -> (o p) f", p=128))

                    def recip_t(rec_t=rec_t, den_t=den_t):
                        nc.vector.reciprocal(rec_t[:], den_t[:])

                    def bcast(rec_t=rec_t, rec_d=rec_d, bc=bc):
                        nc.sync.dma_start(
                            rec_d[:].rearrange("o (p f) -> (o p) f", p=128),
                            rec_t[:])
                        nc.sync.dma_start(bc[:], rec_d[:].partition_broadcast(D))

                    def finish(o_t=o_t, oc=oc, bc=bc, oh=oh, opr=opr):
                        nc.gpsimd.tensor_tensor(o_t[:], oc[0:D, :], bc[:],
                                                mybir.AluOpType.mult)
                        nc.sync.dma_start(
                            ot_d[oh][:, opr * NPAIR:(opr + 1) * NPAIR], o_t[:])

                    deferred = [(2, read_t), (4, recip_t), (6, bcast),
                                (10, finish)]

    nc.compile()
    return nc


def _get_nc():
    if "nc" not in _CACHE:
        _CACHE["nc"] = _build()
    return _CACHE["nc"]


def _make_in_maps(QKV):
    QKV = np.asarray(QKV, dtype=np.float32)
    q = QKV[0].reshape(HEADS, N, D)
    k = QKV[1].reshape(HEADS, N, D)
    v = QKV[2].reshape(HEADS, N, D)
    qt = np.ascontiguousarray(q.transpose(0, 2, 1))      # [16, 64, 4096] f32
    kt = np.ascontiguousarray(k.transpose(0, 2, 1))
    # V' = [V | ones], swizzled to [head, key%128, m-block, d'] so each DMA
    # row is one contiguous 4160-elem run per partition
    vp = np.ones((HEADS, N, D + 1), np.float32)
    vp[:, :, :D] = v
    vp = np.ascontiguousarray(
        vp.reshape(HEADS, MB, 128, D + 1).transpose(0, 2, 1, 3)
    ).astype(ml_dtypes.bfloat16)
    in_maps = []
    for c in range(N_CORES):
        sl = slice(c * HPC, (c + 1) * HPC)
        in_maps.append({
            "qt": qt[sl],
            "kt": kt[sl],
            "v": vp[sl],
        })
    return in_maps


def _assemble(results):
    ot = np.stack([r["ot"] for r in results])            # [8, 2, 64, 4096]
    # the final pair of each core's head 1 was shipped raw: divide here
    for c, r in enumerate(results):
        tail = r["otail"].astype(np.float64)             # [65, 1024]
        ot[c, HPC - 1, :, N - NPAIR:] = (tail[:D] / tail[D:D + 1]).astype(
            np.float32)
    out = ot.reshape(HEADS, D, N).transpose(0, 2, 1)     # [16, 4096, 64]
    return np.ascontiguousarray(out).reshape(B, H, N, D).astype(np.float32)


def kernel(QKV):
    nc = _get_nc()
    res = run_bass_kernel_spmd(nc, _make_in_maps(QKV), list(range(N_CORES)))
    return _assemble(res.results)


# revision 18
# speedup vs baseline: 1.1679x; 1.1679x over previous
"""Trainium2 Bass kernel for nn_BaselineAttention_25984552141259.

Problem: QKV [3, B=2, H=8, N=4096, d=64] fp32 ->
         out[b,h,n,:] = softmax(Q[b,h] @ K[b,h].T) @ V[b,h]

Sharding: B*H = 16 heads, embarrassingly parallel -> 2 heads per core on 8
NeuronCores.

Numerics: scores need fp32 (bf16 Q*K alone costs 2e-2 max-rel-err -- over the
gate), so the S matmul runs fp16 (11-bit mantissa: score err ~0.008 abs). P and V run bf16: the P*V error is tiny
(softmax weights are insensitive to 0.4% V noise) and bf16 halves the exp
write cost + V DMA.

Device algorithm per head (flash-attention style, S^T layout), processed in
chunk-PAIRS of 1024 query columns:

  S^T[m, n]  = sum_d K^T[d, m] * Q^T[d, n]    (PE, fp16 + FWL; K^T padded
                                               d=64 -> 128 partitions ON
                                               DEVICE via one-time pad
                                               tensor_copies -- K=64 matmuls
                                               never un-throttle the PE HAM
                                               clock gate, and unpadded DMA
                                               halves Q/K bytes)
  P^T = exp(S^T - 18)                          (constant bias instead of row
                                               max: scores ~ N(0,64), fp32
                                               psum can't overflow; softmax is
                                               shift-invariant. Split across
                                               TWO engines so neither paces
                                               the PE:
                                               - ACT: spline Exp -> bf16,
                                                 25/32 m-blocks
                                               - DVE: Schraudolph exp2 bit
                                                 trick 7/32:
                                                 bf16_bits = trunc(128*log2e*s
                                                 + C) as int16, bitcast bf16;
                                                 balanced magic constant keeps
                                                 rel err within +-3%)
  O^T[d', n] = sum_m V'[m, d'] * P^T[m, n]     (PE, bf16, accumulate over m;
                                               row d'=64 = softmax denominator
                                               via the host-appended ones col)
  out^T      = O^T[0:64] / bcast(O^T[64])      (DVE copy frees the psum
                                               accumulator in ~1.2us; the
                                               8-cycle/elem DVE reciprocal is
                                               split into 4x256 pieces DEFERRED
                                               into the next pair's DVE stream
                                               so it never stalls the exp
                                               pipeline; denominator broadcast
                                               via DRAM-bounce stride-0 DMA;
                                               final multiply on GPSIMD.
                                               The very last pair ships the
                                               raw [O|den] tile instead and the
                                               host divides during unshard --
                                               saves ~9us of serial drain.)
Host re-transposes out^T -> [N, d] while unsharding.
"""
import numpy as np
import ml_dtypes
from contextlib import ExitStack

import concourse.bass as bass
import concourse.tile as tile
from concourse import bacc, mybir
from concourse.bass_utils import run_bass_kernel_spmd

N_CORES = 8
B, H, N, D = 2, 8, 4096, 64
HEADS = B * H
HPC = HEADS // N_CORES          # heads per core = 2
NCHUNK = 512                    # columns per matmul (ISA moving-dim cap)
NPAIR = 2 * NCHUNK              # chunk-pair processed per stationary reload
PAIRS = N // NPAIR              # 4 pairs per head
MB = N // 128                   # 32 m-blocks of 128 keys
EXP_BIAS = 18.0                 # P = exp(s - 18); s in ~[-60, 60]
LOG2E = 1.4426950408889634

# DVE exp2 bit trick: bf16(2^t) bits ~= 128*(t + 127 - c), c = 0.043 balances
# the (1+f) vs 2^f linear-mantissa error to +-3%; +0.5 compensates the
# truncating fp32->int16 convert. t = (s - EXP_BIAS)*log2e.
TS_SCALE = 128.0 * LOG2E
TS_OFF = 128.0 * (127.0 - 0.043 - EXP_BIAS * LOG2E) + 0.5
# exp stages on DVE instead of ACT; start at m=4 so the pair-boundary copy
# never delays an O-matmul, end before m=30 so the copy isn't queued behind
# a boundary exp
DVE_STAGES = frozenset(m for m in range(MB) if m % 3 == 1 and 4 <= m < 30)

F32 = mybir.dt.float32
F16 = mybir.dt.float16
BF16 = mybir.dt.bfloat16
I16 = mybir.dt.int16

KT_PIECES0 = [1, 1, 2, 4, 8, 16]
KT_PIECES1 = [16, 16]

_CACHE = {}


def _build():
    nc = bacc.Bacc("TRN2", target_bir_lowering=False, debug=False,
                   num_devices=N_CORES)
    qt_d = nc.dram_tensor("qt", [HPC, D, N], F16, kind="ExternalInput").ap()
    kt_d = nc.dram_tensor("kt", [HPC, D, N], F16, kind="ExternalInput").ap()
    v_d = nc.dram_tensor("v", [HPC, 128, MB, D + 1], BF16,
                         kind="ExternalInput").ap()
    ot_d = nc.dram_tensor("ot", [HPC, D, N], F32, kind="ExternalOutput").ap()
    # raw [O | den] of the final pair; host performs that divide
    otail_d = nc.dram_tensor("otail", [D + 1, NPAIR], F32,
                             kind="ExternalOutput").ap()

    with tile.TileContext(nc) as tc, ExitStack() as ctx:
        const = ctx.enter_context(tc.tile_pool(name="const", bufs=1))
        kq = ctx.enter_context(tc.tile_pool(name="kq", bufs=1))
        vpool = ctx.enter_context(tc.tile_pool(name="vpool", bufs=1))
        pexp = ctx.enter_context(tc.tile_pool(name="pexp", bufs=4))
        ocpool = ctx.enter_context(tc.tile_pool(name="ocpool", bufs=2))
        opool = ctx.enter_context(tc.tile_pool(name="opool", bufs=2))
        bcpool = ctx.enter_context(tc.tile_pool(name="bcpool", bufs=2))
        rpool = ctx.enter_context(tc.tile_pool(name="rpool", bufs=2))
        s_ps = ctx.enter_context(tc.tile_pool(name="s_ps", bufs=3, space="PSUM"))
        ot_ps = ctx.enter_context(tc.tile_pool(name="ot_ps", bufs=1, space="PSUM"))
        rdram = ctx.enter_context(tc.tile_pool(name="rdram", bufs=2, space="DRAM"))

        bias_t = const.tile([128, 1], F32)
        nc.vector.memset(bias_t[:], -EXP_BIAS)

        # ---- loads ----
        kt_tiles = [[None] * MB for _ in range(HPC)]
        v_tiles = [[None] * MB for _ in range(HPC)]
        qt_tiles = [[None] * PAIRS for _ in range(HPC)]

        def load_kt(h, m0, g, eng=None):
            t = kq.tile([128, g, 128], F16, tag=f"kt_{h}_{m0}",
                        name=f"kt_{h}_{m0}")
            nc.vector.memset(t[D:128, :, :], 0.0)
            (eng or nc.gpsimd).dma_start(
                t[0:D, :, :],
                kt_d[h, :, m0 * 128:(m0 + g) * 128].rearrange(
                    "p (t q) -> p t q", q=128))
            for j in range(g):
                kt_tiles[h][m0 + j] = t[:, j, :]

        def load_qt(h, pr, split=1, engs=None):
            t = kq.tile([128, 2, NCHUNK], F16, tag=f"qt_{h}_{pr}",
                        name=f"qt_{h}_{pr}")
            nc.vector.memset(t[D:128, :, :], 0.0)
            w = NPAIR // split
            for i in range(split):
                eng = (engs or [nc.gpsimd])[i % len(engs or [nc.gpsimd])]
                eng.dma_start(
                    t[0:D, :, :].rearrange("p t q -> p (t q)")[:,
                        i * w:(i + 1) * w].rearrange("p (t q) -> p t q", q=w),
                    qt_d[h, :, pr * NPAIR + i * w:pr * NPAIR + (i + 1) * w
                         ].rearrange("p (t q) -> p t q", q=w))
            qt_tiles[h][pr] = t

        def load_v(h, m0, g, eng=None):
            t = vpool.tile([128, g, D + 1], BF16, tag=f"v_{h}_{m0}",
                           name=f"v_{h}_{m0}")
            (eng or nc.gpsimd).dma_start(t[:], v_d[h, :, m0:m0 + g, :])
            for j in range(g):
                v_tiles[h][m0 + j] = t[:, j, :]

        with nc.named_scope("load"):
            # first wave across four trigger engines: the first m-block,
            # q-pair and v-block land in parallel DMA queues within ~3us
            load_kt(0, 0, 1, eng=nc.gpsimd)
            load_qt(0, 0, split=4, engs=[nc.sync, nc.scalar])
            load_v(0, 0, 1, eng=nc.sync)
            load_kt(0, 1, 1, eng=nc.scalar)
            m0 = 2
            for i, g in enumerate([2, 4, 8, 16]):
                load_kt(0, m0, g)
                load_v(0, m0 - 1, g)        # v trails kt by one piece
                if i == 0:
                    load_qt(0, 1, split=2, engs=[nc.sync, nc.scalar])
                if i == 1:
                    load_qt(0, 2)
                if i == 2:
                    load_qt(0, 3)
                m0 += g
            load_v(0, m0 - 1, 1)
            m0 = 0
            for i, g in enumerate(KT_PIECES1):
                load_kt(1, m0, g)
                load_v(1, m0, g)
                load_qt(1, 2 * i)
                load_qt(1, 2 * i + 1)
                m0 += g

        # ---- compute ----
        # deferred normalize actions from the previous pair, keyed by the
        # stage index of the CURRENT pair at which each may be issued
        deferred = []

        def run_deferred(stage):
            while deferred and deferred[0][0] <= stage:
                deferred.pop(0)[1]()

        for h in range(HPC):
            with nc.named_scope(f"head{h}"):
                for pr in range(PAIRS):
                    qt_t = qt_tiles[h][pr]
                    last = (h == HPC - 1) and (pr == PAIRS - 1)
                    ot_t = ot_ps.tile([D + 1, 2, NCHUNK], F32, tag="ot",
                                      name=f"ot_{h}_{pr}")
                    for m in range(MB):
                        run_deferred(m)
                        s_t = s_ps.tile([128, 2, NCHUNK], F32, tag="s")
                        for j in range(2):
                            nc.tensor.matmul(s_t[:, j, :], kt_tiles[h][m],
                                             qt_t[:, j, :],
                                             start=True, stop=True)
                        p_t = pexp.tile([128, 2, NCHUNK], BF16, tag="p")
                        if m in DVE_STAGES:
                            nc.vector.tensor_scalar(
                                p_t[:].bitcast(I16), s_t[:],
                                TS_SCALE, TS_OFF,
                                mybir.AluOpType.mult, mybir.AluOpType.add)
                        else:
                            nc.scalar.activation(
                                p_t[:], s_t[:],
                                mybir.ActivationFunctionType.Exp,
                                bias=bias_t[:], scale=1.0)
                        for j in range(2):
                            nc.tensor.matmul(ot_t[:, j, :], v_tiles[h][m],
                                             p_t[:, j, :],
                                             start=(m == 0),
                                             stop=(m == MB - 1))
                    run_deferred(MB)

                    # free the single psum accumulator ASAP via a DVE copy
                    oc = ocpool.tile([D + 1, NPAIR], F32, tag="oc")
                    nc.vector.tensor_copy(
                        oc[:], ot_t[:].rearrange("p t q -> p (t q)"))
                    if last:
                        # raw [O | den] out; host divides during unshard
                        nc.sync.dma_start(otail_d[:, :], oc[:])
                        continue
                    # Reciprocal trick: the DVE iterative divide is ~8
                    # cycles per FREE element but parallel across
                    # partitions, so 1/den on the [1, 1024] row costs 6.5us
                    # while the same values transposed to [128, 8] cost
                    # ~0.3us. The DRAM bounce (needed for the partition
                    # broadcast anyway) provides the transpose for free.
                    # Everything below is deferred into the next pair's
                    # stream and runs off the critical path.
                    bc = bcpool.tile([D, NPAIR], F32, tag="bc")
                    o_t = opool.tile([D, NPAIR], F32, tag="o")
                    den_d = rdram.tile([1, NPAIR], F32, tag="den_d")
                    rec_d = rdram.tile([1, NPAIR], F32, tag="rec_d")
                    den_t = rpool.tile([128, NPAIR // 128], F32, tag="den_t")
                    rec_t = rpool.tile([128, NPAIR // 128], F32, tag="rec_t")
                    nc.sync.dma_start(den_d[:], oc[D:D + 1, :])
                    oh, opr = h, pr

                    def read_t(den_t=den_t, den_d=den_d):
                        nc.sync.dma_start(
                            den_t[:],
                            den_d[:].rearrange("o (p f) -> (o p) f", p=128))

                    def recip_t(rec_t=rec_t, den_t=den_t):
                        nc.vector.reciprocal(rec_t[:], den_t[:])

                    def bcast(rec_t=rec_t, rec_d=rec_d, bc=bc):
                        nc.sync.dma_start(
                            rec_d[:].rearrange("o (p f) -> (o p) f", p=128),
                            rec_t[:])
                        nc.sync.dma_start(bc[:], rec_d[:].partition_broadcast(D))

                    def finish(o_t=o_t, oc=oc, bc=bc, oh=oh, opr=opr):
                        nc.gpsimd.tensor_tensor(o_t[:], oc[0:D, :], bc[:],
                                                mybir.AluOpType.mult)
                        nc.sync.dma_start(
                            ot_d[oh][:, opr * NPAIR:(opr + 1) * NPAIR], o_t[:])

                    deferred = [(2, read_t), (4, recip_t), (6, bcast),
                                (10, finish)]

    nc.compile()
    return nc


def _get_nc():
    if "nc" not in _CACHE:
        _CACHE["nc"] = _build()
    return _CACHE["nc"]


def _make_in_maps(QKV):
    QKV = np.asarray(QKV, dtype=np.float32)
    q = QKV[0].reshape(HEADS, N, D)
    k = QKV[1].reshape(HEADS, N, D)
    v = QKV[2].reshape(HEADS, N, D)
    qt = np.ascontiguousarray(q.transpose(0, 2, 1)).astype(np.float16)
    kt = np.ascontiguousarray(k.transpose(0, 2, 1)).astype(np.float16)
    # V' = [V | ones], swizzled to [head, key%128, m-block, d'] so each DMA
    # row is one contiguous 4160-elem run per partition
    vp = np.ones((HEADS, N, D + 1), np.float32)
    vp[:, :, :D] = v
    vp = np.ascontiguousarray(
        vp.reshape(HEADS, MB, 128, D + 1).transpose(0, 2, 1, 3)
    ).astype(ml_dtypes.bfloat16)
    in_maps = []
    for c in range(N_CORES):
        sl = slice(c * HPC, (c + 1) * HPC)
        in_maps.append({
            "qt": qt[sl],
            "kt": kt[sl],
            "v": vp[sl],
        })
    return in_maps


def _assemble(results):
    ot = np.stack([r["ot"] for r in results])            # [8, 2, 64, 4096]
    # the final pair of each core's head 1 was shipped raw: divide here
    for c, r in enumerate(results):
        tail = r["otail"].astype(np.float64)             # [65, 1024]
        ot[c, HPC - 1, :, N - NPAIR:] = (tail[:D] / tail[D:D + 1]).astype(
            np.float32)
    out = ot.reshape(HEADS, D, N).transpose(0, 2, 1)     # [16, 4096, 64]
    return np.ascontiguousarray(out).reshape(B, H, N, D).astype(np.float32)


def kernel(QKV):
    nc = _get_nc()
    res = run_bass_kernel_spmd(nc, _make_in_maps(QKV), list(range(N_CORES)))
    return _assemble(res.results)
